# revision 4
# baseline (speedup 1.0000x reference)
"""LoRA-MoE fused kernel for 8x Trainium2 NeuronCores (Bass/Tile).

Math (per batch sample b, data-parallel across 8 cores):
    g_b    = gate_w @ mean_s(x_b) + gate_b                      # [E]
    out_b  = x_b @ W^T + ((x_b @ A^T) * g_rep) @ Bt + bias      # [S, D_OUT]
where A = lora_A reshaped [E*R, D_IN], Bt[(e,r), o] = lora_B[e, o, r],
g_rep[(e,r)] = g_b[e].  The merged per-sample weights of the reference
(W + sum_e g[b,e] * lora_B[e] @ lora_A[e]) are never materialized.

On-chip orientation: out^T tiles [o_tile=128 part, s_chunk=512 free];
contraction (D_IN or E*R) lives on partitions.  Heavy tensors (x, W, A,
Bt, u, out) are bfloat16: PE streams at the same 1 cycle/row as f32r
but the per-matmul LDWEIGHTS drops from ~187 ns (f32r, not hidden
under the 213 ns stream) to ~53 ns (hidden), and DMA bytes halve,
which pulls the first x chunk in ~11 us instead of ~23 us.  The gate
chain stays f32.  Main loop is i-chunk-outer / s-chunk-inner so
consecutive matmuls cycle across 4 PSUM banks and share stationary
weights.  Base + LoRA matmuls accumulate into one PSUM group; bias is
added during the PSUM->SBUF copy, alternating between Vector and
Scalar engines.  DMA: x chunks go first on the sync queue; AT/gw/Bt
triggers sit after phase-0/1 scalar work so their HBM traffic lands
after x; output stores alternate sync/gpsimd queues.
"""

import sys

import numpy as np

try:
    import concourse.bass  # noqa: F401
except ImportError:  # pragma: no cover - fallback for bare environments
    for _p in (
        "/root/.axon_site",
        "/root/.axon_site/_ro/trn_rl_repo",
        "/root/.axon_site/_ro/pypackages",
        "/opt/trn_rl_repo",
    ):
        if _p not in sys.path:
            sys.path.append(_p)

import ml_dtypes
import concourse.bass as bass  # noqa: F401
import concourse.mybir as mybir
import concourse.tile as tile
from concourse import bacc, bass_utils

S, B, D_IN, D_OUT, E, R = 2048, 8, 1024, 4096, 8, 16
NCORES = 8
ER = E * R            # 128 (one partition dim worth of lora rows)
KC = D_IN // 128      # 8 contraction chunks
NOT = D_OUT // 128    # 32 output tiles
SC = 512              # s-chunk (one PSUM bank of f32)
NSC = S // SC         # 4

F32 = mybir.dt.float32
BF16 = mybir.dt.bfloat16

Ident = mybir.ActivationFunctionType.Identity
CopyF = mybir.ActivationFunctionType.Copy

BF16NP = ml_dtypes.bfloat16


def _build_nc(n_cores: int = NCORES):
    nc = bacc.Bacc(
        "TRN2", target_bir_lowering=False, debug=False, num_devices=n_cores
    )

    xT = nc.dram_tensor("xT", [D_IN, S], BF16, kind="ExternalInput").ap()
    WTb = nc.dram_tensor("WTb", [NOT, 128, D_IN], BF16, kind="ExternalInput").ap()
    AT = nc.dram_tensor("AT", [128, KC, ER], BF16, kind="ExternalInput").ap()
    Bt = nc.dram_tensor("Bt", [ER, D_OUT], BF16, kind="ExternalInput").ap()
    gwT = nc.dram_tensor("gwT", [128, KC, ER], F32, kind="ExternalInput").ap()
    gb = nc.dram_tensor("gb", [ER, 1], F32, kind="ExternalInput").ap()
    bias_t = nc.dram_tensor("bias_t", [128, NOT], F32, kind="ExternalInput").ap()
    outT = nc.dram_tensor("outT", [D_OUT, S], BF16, kind="ExternalOutput").ap()

    NDEFER = 4  # leading o_tiles processed base-only; lora added later

    with (
        tile.TileContext(nc) as tc,
        tc.tile_pool(name="singles", bufs=1) as singles,
        tc.tile_pool(name="wpool", bufs=6) as wpool,
        tc.tile_pool(name="opool", bufs=3) as opool,
        tc.tile_pool(name="odefer", bufs=NDEFER) as odefer,
        tc.tile_pool(name="ps_a", bufs=4, space="PSUM") as ps_a,
        tc.tile_pool(name="ps_b", bufs=4, space="PSUM") as ps_b,
    ):
        # ---- x^T (stays resident; sync queue carries x first, then the
        # output stores) + per-chunk column sums for the gate; reduces
        # split across Vector and Scalar engines
        x_sb = singles.tile([128, KC, S], BF16)
        xsum = singles.tile([128, KC], F32)
        scratch = singles.tile([128, S], BF16)
        x_r = xT.rearrange("(c p) s -> c p s", p=128)

        _wt_cache = {}

        def wt_load(ot):
            if ot in _wt_cache:
                return _wt_cache.pop(ot)
            wt = wpool.tile([128, KC, 128], BF16, tag="wt")
            nc.scalar.dma_start(
                out=wt[:], in_=WTb[ot].rearrange("p (c m) -> p c m", c=KC)
            )
            return wt

        # prefetch the first wt tiles before anything else lands on the
        # scalar engine's queue, so their HWDGE triggers fire immediately
        for _ot in range(4):
            wt = wpool.tile([128, KC, 128], BF16, tag="wt")
            nc.scalar.dma_start(
                out=wt[:], in_=WTb[_ot].rearrange("p (c m) -> p c m", c=KC)
            )
            _wt_cache[_ot] = wt

        # x is split across the sync and gpsimd queues (even/odd chunks) so
        # consecutive chunks stream concurrently instead of FIFO-serializing
        # behind each other; the tiny gate/bias inputs go first on gpsimd
        gb_sb = singles.tile([128, 1], F32)
        nc.gpsimd.dma_start(out=gb_sb[:], in_=gb)
        bias_sb = singles.tile([128, NOT], F32)
        nc.gpsimd.dma_start(out=bias_sb[:], in_=bias_t)
        for c in range(KC):
            eng = nc.sync if c % 2 == 0 else nc.gpsimd
            eng.dma_start(out=x_sb[:, c, :], in_=x_r[c])

        def base_mms(accs, wt, stop):
            for c in range(KC):
                for sc in range(NSC):
                    nc.tensor.matmul(
                        accs[sc][:],
                        wt[:, c, :],
                        x_sb[:, c, sc * SC : (sc + 1) * SC],
                        start=(c == 0),
                        stop=(stop and c == KC - 1),
                    )

        def bias_copy(o_sb, accs, ot):
            for sc in range(NSC):
                sl = slice(sc * SC, (sc + 1) * SC)
                if (ot + sc) % 2 == 0:
                    nc.vector.tensor_scalar_add(
                        o_sb[:, sl], accs[sc][:], bias_sb[:, ot : ot + 1]
                    )
                else:
                    nc.scalar.activation(
                        out=o_sb[:, sl],
                        in_=accs[sc][:],
                        func=Ident,
                        bias=bias_sb[:, ot : ot + 1],
                        scale=1.0,
                    )

        _ps_toggle = [0]

        def psum_group():
            pool = ps_a if _ps_toggle[0] % 2 == 0 else ps_b
            _ps_toggle[0] += 1
            accs = []
            for _sc in range(NSC):
                acc = pool.tile([128, SC], F32, tag="acc")
                accs.append(acc)
            return accs

        # ---- deferred o_tiles: base-only matmuls, K-split into chunk halves
        # (c 0-3 and c 4-7) so PSUM banks recycle mid-load and the PE always
        # has dense work while x streams in.  Phase order: ot0/ot1 over the
        # first chunk half (DMA-gated), ot2/ot3 over the same chunks (dense),
        # then the second halves.  A-half carries the bias; B-half is added.
        defer_o = []
        defer_wt = []
        for ot in range(NDEFER):
            defer_wt.append(wt_load(ot))
            o_sb = odefer.tile([128, S], BF16, tag="od")
            defer_o.append(o_sb)
        KH = KC // 2
        for phase, (ots, c_lo, c_hi) in enumerate(
            [
                ((0, 1), 0, KH),
                ((2, 3), 0, KH),
                ((0, 1), KH, KC),
                ((2, 3), KH, KC),
            ]
        ):
            groups = {}
            for ot in ots:
                groups[ot] = psum_group()
            # chunk-major across the ot pair: 8 ready matmuls per arriving
            # x chunk instead of 4 (the PE stream is in-order)
            for c in range(c_lo, c_hi):
                for ot in ots:
                    for sc in range(NSC):
                        nc.tensor.matmul(
                            groups[ot][sc][:],
                            defer_wt[ot][:, c, :],
                            x_sb[:, c, sc * SC : (sc + 1) * SC],
                            start=(c == c_lo),
                            stop=(c == c_hi - 1),
                        )
            for ot in ots:
                if c_lo == 0:
                    bias_copy(defer_o[ot], groups[ot], ot)
                else:
                    for sc in range(NSC):
                        sl = slice(sc * SC, (sc + 1) * SC)
                        nc.vector.tensor_add(
                            defer_o[ot][:, sl], defer_o[ot][:, sl], groups[ot][sc][:]
                        )
            if phase == 0:
                # at/gw are needed only once all of x has landed; their
                # triggers sit behind phase-0's scalar bias work so the
                # early HBM window stays clear for x
                at_sb = singles.tile([128, KC, ER], BF16)
                nc.scalar.dma_start(out=at_sb[:], in_=AT[:])
                gw_sb = singles.tile([128, KC, ER], F32)
                nc.scalar.dma_start(out=gw_sb[:], in_=gwT[:])
            elif phase == 1:
                bt_sb = singles.tile([128, D_OUT], BF16)
                nc.scalar.dma_start(out=bt_sb[:], in_=Bt)

        for c in range(KC):
            if c % 2 == 0:
                nc.vector.reduce_sum(
                    out=xsum[:, c : c + 1],
                    in_=x_sb[:, c, :],
                    axis=mybir.AxisListType.X,
                )
            else:
                nc.scalar.activation(
                    out=scratch[:],
                    in_=x_sb[:, c, :],
                    func=CopyF,
                    accum_out=xsum[:, c : c + 1],
                )

        # ---- u^T[er, s] = A @ x_b^T  (needs all of x, only PE + copies)
        u_sb = singles.tile([128, S], BF16)
        uaccs = psum_group()
        for c in range(KC):
            for sc in range(NSC):
                nc.tensor.matmul(
                    uaccs[sc][:],
                    at_sb[:, c, :],
                    x_sb[:, c, sc * SC : (sc + 1) * SC],
                    start=(c == 0),
                    stop=(c == KC - 1),
                )
        for sc in range(NSC):
            nc.vector.tensor_copy(u_sb[:, sc * SC : (sc + 1) * SC], uaccs[sc][:])

        # ---- gate: g128[er] = sum_c gw_sb[:,c,:]^T @ xsum[:,c] + gb
        # (gwT is pre-scaled by 1/S on the host, so xsum acts as the mean)
        g_ps = ps_b.tile([128, 1], F32, tag="acc")
        for c in range(KC):
            nc.tensor.matmul(
                g_ps[:],
                gw_sb[:, c, :],
                xsum[:, c : c + 1],
                start=(c == 0),
                stop=(c == KC - 1),
            )
        g_sb = singles.tile([128, 1], F32)
        nc.vector.tensor_add(g_sb[:], g_ps[:], gb_sb[:])

        # fold the gate into Bt: bts[er, o] = g[er] * Bt[er, o],
        # split across Vector and Scalar engines
        bts_sb = singles.tile([128, D_OUT], BF16)
        half = D_OUT // 2
        nc.vector.tensor_scalar_mul(bts_sb[:, :half], bt_sb[:, :half], g_sb[:])
        nc.scalar.activation(
            out=bts_sb[:, half:],
            in_=bt_sb[:, half:],
            func=Ident,
            scale=g_sb[:],
        )

        def store(o_sb, ot, sc=None):
            # whole-tile stores alternate queues by tile; the chunked tail
            # stores alternate by chunk so their ~650 ns trigger
            # instructions run on two engines in parallel
            eng = nc.sync if (ot if sc is None else sc) % 2 == 0 else nc.gpsimd
            osl = slice(ot * 128, (ot + 1) * 128)
            if sc is None:
                eng.dma_start(out=outT[osl, :], in_=o_sb[:])
            else:
                sl = slice(sc * SC, (sc + 1) * SC)
                eng.dma_start(out=outT[osl, sl], in_=o_sb[:, sl])

        # ---- lora for the deferred o_tiles, then store them
        for ot in range(NDEFER):
            osl = slice(ot * 128, (ot + 1) * 128)
            laccs = psum_group()
            for sc in range(NSC):
                nc.tensor.matmul(
                    laccs[sc][:],
                    bts_sb[:, osl],
                    u_sb[:, sc * SC : (sc + 1) * SC],
                    start=True,
                    stop=True,
                )
            for sc in range(NSC):
                sl = slice(sc * SC, (sc + 1) * SC)
                nc.vector.tensor_add(
                    defer_o[ot][:, sl], defer_o[ot][:, sl], laccs[sc][:]
                )
            store(defer_o[ot], ot)

        # ---- steady-state fused loop
        for ot in range(NDEFER, NOT):
            wt = wt_load(ot)
            o_sb = opool.tile([128, S], BF16, tag="o")
            osl = slice(ot * 128, (ot + 1) * 128)
            accs = psum_group()
            base_mms(accs, wt, stop=False)
            for sc in range(NSC):
                nc.tensor.matmul(
                    accs[sc][:],
                    bts_sb[:, osl],
                    u_sb[:, sc * SC : (sc + 1) * SC],
                    start=False,
                    stop=True,
                )
            bias_copy(o_sb, accs, ot)
            if ot >= NOT - 2:
                for sc in range(NSC):
                    store(o_sb, ot, sc)
            else:
                store(o_sb, ot)

    nc.compile()
    return nc


def _prep_in_maps(x, gate_w, gate_b, W, bias, lora_A, lora_B):
    f32 = np.float32
    x = np.asarray(x, f32)
    gate_w = np.asarray(gate_w, f32)
    gate_b = np.asarray(gate_b, f32)
    W = np.asarray(W, f32)
    bias = np.asarray(bias, f32)
    lora_A = np.asarray(lora_A, f32)
    lora_B = np.asarray(lora_B, f32)

    WTb = np.ascontiguousarray(
        W.reshape(NOT, 128, KC, 128).transpose(0, 3, 2, 1).reshape(NOT, 128, D_IN)
    ).astype(BF16NP)
    AT = np.ascontiguousarray(
        lora_A.reshape(ER, D_IN).T.reshape(KC, 128, ER).transpose(1, 0, 2)
    ).astype(BF16NP)
    Bt = np.ascontiguousarray(lora_B.transpose(0, 2, 1).reshape(ER, D_OUT)).astype(
        BF16NP
    )
    gwT = np.ascontiguousarray(
        (np.repeat(gate_w, R, axis=0).T / np.float32(S))
        .reshape(KC, 128, ER)
        .transpose(1, 0, 2)
    )
    gbr = np.ascontiguousarray(np.repeat(gate_b, R).reshape(ER, 1))
    bias_t = np.ascontiguousarray(bias.reshape(NOT, 128).T)

    shared = {
        "WTb": WTb,
        "AT": AT,
        "Bt": Bt,
        "gwT": gwT.astype(f32),
        "gb": gbr,
        "bias_t": bias_t,
    }
    in_maps = []
    for b in range(NCORES):
        m = dict(shared)
        m["xT"] = np.ascontiguousarray(x[:, b, :].T).astype(BF16NP)
        in_maps.append(m)
    return in_maps


def run(inputs, trace=False, trace_cores=None):
    """Build + run on 8 cores. Returns (out [S,B,D_OUT], BassKernelResults)."""
    in_maps = _prep_in_maps(**inputs)
    nc = _build_nc()
    kwargs = {}
    if trace:
        _register_axon_ntff_hook()
        kwargs = dict(trace=True, trace_cores=trace_cores or [0])
    res = bass_utils.run_bass_kernel_spmd(
        nc, in_maps, core_ids=list(range(NCORES)), **kwargs
    )
    out = np.empty((S, B, D_OUT), np.float32)
    for b in range(NCORES):
        out[:, b, :] = res.results[b]["outT"].T.astype(np.float32)
    return out, res


def _register_axon_ntff_hook():
    """antenv.axon_hooks is missing on this image; synthesize it so
    run_bass_kernel_spmd(trace=True) can reach the axon NTFF profiler."""
    import types

    try:
        from antenv.axon_hooks import get_axon_ntff_profile_hook  # noqa: F401

        return  # real module present
    except ImportError:
        pass
    try:
        from trn_agent_boot.trn_boot import _ntff_profile_via_ctypes
    except ImportError:
        return
    import antenv

    mod = types.ModuleType("antenv.axon_hooks")
    _state = {"hook": None}
    mod.set_axon_ntff_profile_hook = lambda h: _state.__setitem__("hook", h)
    mod.get_axon_ntff_profile_hook = lambda: _state["hook"]
    sys.modules["antenv.axon_hooks"] = mod
    antenv.axon_hooks = mod
    hook = _ntff_profile_via_ctypes("/opt/axon/libaxon_pjrt.so")
    if hook is not None:
        mod.set_axon_ntff_profile_hook(hook)


def kernel(**inputs) -> np.ndarray:
    out, _ = run(inputs, trace=False)
    return out
